# revision 3
# baseline (speedup 1.0000x reference)
"""Adaptive softmax (head + 2 projected tails) CE loss on 8 TRN2 NeuronCores.

Strategy: pure data parallelism over tokens (4096 tokens -> 512/core), weights
replicated. Per core:
  - head logits  x @ Wh        [512,1024]@[1024,20002]
  - tail0 logits (x@P0) @ W0   via h0T = P0^T x^T, then h0T^T slices as lhsT
  - tail1 logits (x@P1) @ W1
  - lse per token per segment: fused Exp+row-sum on the scalar engine
    (activation accum_out), accumulated across vocab supertiles
  - picked logit per token: host gathers the label's weight column (index
    prep), device computes dot(x_n, W[:,lab_n]) via tensor_tensor_reduce
  - loss = ln(sumexp) - picked, tails masked by label-range masks
All matmuls in bf16 (inputs cast on host), accumulation fp32 in PSUM.
"""

import sys

for _p in ("/opt/trn_rl_repo",):
    if _p not in sys.path:
        sys.path.insert(0, _p)

import numpy as np
import ml_dtypes

BF16 = ml_dtypes.bfloat16

# ---- problem constants (hardcoded per spec) ----
B, S, H = 8, 512, 1024
N = B * S                      # 4096 tokens
NCORES = 8
TOK = N // NCORES              # 512 tokens per core
TT = TOK // 128                # 4 token tiles
KH = H // 128                  # 8 contraction chunks for H
V_HEAD = 20002
V_TAIL = 20000
P0, K0 = 256, 2                # tail0 proj dim
P1 = 64                        # tail1 proj dim
CUT0, CUT1 = 20000, 40000
SUP = 2048                     # vocab supertile width (4 PSUM banks)


def _supertiles(v):
    out = []
    base = 0
    while base < v:
        w = min(SUP, v - base)
        out.append((base, w))
        base += w
    return out


SUPS_HEAD = _supertiles(V_HEAD)   # 9x2048 + 1570
SUPS_TAIL = _supertiles(V_TAIL)   # 9x2048 + 1568
NSUP = len(SUPS_HEAD)             # 10 (same count for tails)
assert len(SUPS_TAIL) == NSUP

_NC_CACHE = None


def _build_nc():
    import concourse.bass as bass
    import concourse.tile as tile
    from concourse import bacc, mybir

    f32 = mybir.dt.float32
    bf16 = mybir.dt.bfloat16
    Act = mybir.ActivationFunctionType
    Alu = mybir.AluOpType

    nc = bacc.Bacc("TRN2", target_bir_lowering=False, debug=False)

    # inputs (per-core shards / replicated weights)
    d_xT = nc.dram_tensor("xT", [H, TOK], bf16, kind="ExternalInput")
    d_p0 = nc.dram_tensor("p0", [H, P0], bf16, kind="ExternalInput")
    d_p1 = nc.dram_tensor("p1", [H, P1], bf16, kind="ExternalInput")
    d_x = nc.dram_tensor("x", [TOK, H], bf16, kind="ExternalInput")
    d_wg = nc.dram_tensor("wg", [TOK, H], bf16, kind="ExternalInput")
    d_w0g = nc.dram_tensor("w0g", [TOK, P0], bf16, kind="ExternalInput")
    d_w1g = nc.dram_tensor("w1g", [TOK, P1], bf16, kind="ExternalInput")
    d_m0 = nc.dram_tensor("m0", [128, TT], f32, kind="ExternalInput")
    d_m1 = nc.dram_tensor("m1", [128, TT], f32, kind="ExternalInput")
    d_wh = nc.dram_tensor("wh", [H, V_HEAD], bf16, kind="ExternalInput")
    d_w0 = nc.dram_tensor("w0", [P0, V_TAIL], bf16, kind="ExternalInput")
    d_w1 = nc.dram_tensor("w1", [P1, V_TAIL], bf16, kind="ExternalInput")
    # out[p, seg*TT + t] = loss of token t*128+p for segment seg (0=t0,1=t1,2=head)
    d_out = nc.dram_tensor("out", [128, 3 * TT], f32, kind="ExternalOutput")

    with tile.TileContext(nc) as tc:
        with (
            tc.tile_pool(name="sing", bufs=1) as sing,
            tc.tile_pool(name="wst", bufs=2) as wst,
            tc.tile_pool(name="psum", bufs=2, space="PSUM") as psum,
            tc.tile_pool(name="scr", bufs=2) as scr,
        ):
            # ---- resident SBUF tensors ----
            xT_sb = sing.tile([128, KH, TOK], bf16)
            p0_sb = sing.tile([128, KH, P0], bf16)
            p1_sb = sing.tile([128, KH, P1], bf16)
            nc.sync.dma_start(out=xT_sb[:, :, :], in_=d_xT.ap().rearrange("(k p) t -> p k t", p=128))
            nc.sync.dma_start(out=p0_sb[:, :, :], in_=d_p0.ap().rearrange("(k p) c -> p k c", p=128))
            nc.sync.dma_start(out=p1_sb[:, :, :], in_=d_p1.ap().rearrange("(k p) c -> p k c", p=128))

            x_sb = sing.tile([128, TT, H], bf16)
            wg_sb = sing.tile([128, TT, H], bf16)
            w0g_sb = sing.tile([128, TT, P0], bf16)
            w1g_sb = sing.tile([128, TT, P1], bf16)
            m0_sb = sing.tile([128, TT], f32)
            m1_sb = sing.tile([128, TT], f32)
            nc.sync.dma_start(out=x_sb[:, :, :], in_=d_x.ap().rearrange("(t p) h -> p t h", p=128))
            nc.sync.dma_start(out=wg_sb[:, :, :], in_=d_wg.ap().rearrange("(t p) h -> p t h", p=128))
            nc.sync.dma_start(out=w0g_sb[:, :, :], in_=d_w0g.ap().rearrange("(t p) c -> p t c", p=128))
            nc.sync.dma_start(out=w1g_sb[:, :, :], in_=d_w1g.ap().rearrange("(t p) c -> p t c", p=128))
            nc.sync.dma_start(out=m0_sb[:, :], in_=d_m0.ap()[:, :])
            nc.sync.dma_start(out=m1_sb[:, :], in_=d_m1.ap()[:, :])

            h0T_sb = sing.tile([128, K0, TOK], bf16)
            h1T_sb = sing.tile([128, TOK], bf16)      # only partitions 0:64 used
            h0_sb = sing.tile([128, TT, P0], bf16)    # token-major, for picked
            h1_sb = sing.tile([128, TT, P1], bf16)

            acc = sing.tile([128, 3, TT, NSUP], f32)  # exp-sum partials
            picked = sing.tile([128, 3 * TT], f32)
            sums = sing.tile([128, 3 * TT], f32)
            lnS = sing.tile([128, 3 * TT], f32)
            loss = sing.tile([128, 3 * TT], f32)

            # ---- h0T = P0^T @ x^T  [256, 512] ; h1T = P1^T @ x^T [64, 512] ----
            for c in range(K0):
                pt = psum.tile([128, SUP], f32, tag="pt")
                for k in range(KH):
                    nc.tensor.matmul(
                        pt[:, 0:TOK],
                        lhsT=p0_sb[:, k, c * 128:(c + 1) * 128],
                        rhs=xT_sb[:, k, :],
                        start=(k == 0), stop=(k == KH - 1),
                    )
                nc.vector.tensor_copy(h0T_sb[:, c, :], pt[:, 0:TOK])
            pt = psum.tile([128, SUP], f32, tag="pt")
            for k in range(KH):
                nc.tensor.matmul(
                    pt[0:P1, 0:TOK],
                    lhsT=p1_sb[:, k, :],
                    rhs=xT_sb[:, k, :],
                    start=(k == 0), stop=(k == KH - 1),
                )
            nc.vector.tensor_copy(h1T_sb[0:P1, :], pt[0:P1, 0:TOK])

            # ---- token-major h0 [tok, 256] / h1 [tok, 64] for picked-tail dots ----
            pt0 = psum.tile([128, SUP], f32, tag="pt")
            for t in range(TT):
                for k in range(KH):
                    nc.tensor.matmul(
                        pt0[:, t * P0:(t + 1) * P0],
                        lhsT=xT_sb[:, k, t * 128:(t + 1) * 128],
                        rhs=p0_sb[:, k, :],
                        start=(k == 0), stop=(k == KH - 1),
                    )
            pt1 = psum.tile([128, SUP], f32, tag="pt")
            for t in range(TT):
                for k in range(KH):
                    nc.tensor.matmul(
                        pt1[:, t * P1:(t + 1) * P1],
                        lhsT=xT_sb[:, k, t * 128:(t + 1) * 128],
                        rhs=p1_sb[:, k, :],
                        start=(k == 0), stop=(k == KH - 1),
                    )
            # picked dots: seg 0 (tail0), seg 1 (tail1), seg 2 (head)
            for t in range(TT):
                nc.vector.tensor_copy(h0_sb[:, t, :], pt0[:, t * P0:(t + 1) * P0])
                nc.vector.tensor_copy(h1_sb[:, t, :], pt1[:, t * P1:(t + 1) * P1])
            for t in range(TT):
                for seg, (hs, ws, width) in enumerate((
                    (h0_sb, w0g_sb, P0),
                    (h1_sb, w1g_sb, P1),
                    (x_sb, wg_sb, H),
                )):
                    sc = scr.tile([128, H], bf16, tag="ttr")
                    nc.vector.tensor_mul(sc[:, 0:width], hs[:, t, :], ws[:, t, :])
                    nc.vector.tensor_reduce(
                        out=picked[:, seg * TT + t: seg * TT + t + 1],
                        in_=sc[:, 0:width],
                        axis=mybir.AxisListType.X, op=Alu.add,
                    )

            # ---- main vocab loops: matmul supertile -> fused exp+rowsum ----
            def seg_loop(seg, d_w, nk, sups, lhsT_of, kparts):
                w_r = d_w.ap().rearrange("(k p) v -> p k v", p=kparts)
                for s, (base, w) in enumerate(sups):
                    wt = wst.tile([kparts, nk, SUP], bf16, tag=f"w{seg}")
                    nc.sync.dma_start(out=wt[:, :, 0:w], in_=w_r[:, :, base:base + w])
                    for t in range(TT):
                        pt = psum.tile([128, SUP], f32, tag="pt")
                        nb = 0
                        while nb < w:
                            nw = min(512, w - nb)
                            for k in range(nk):
                                nc.tensor.matmul(
                                    pt[:, nb:nb + nw],
                                    lhsT=lhsT_of(k, t),
                                    rhs=wt[0:kparts, k, nb:nb + nw],
                                    start=(k == 0), stop=(k == nk - 1),
                                )
                            nb += nw
                        ex = scr.tile([128, SUP], bf16, tag="exp")
                        nc.scalar.activation(
                            out=ex[:, 0:w], in_=pt[:, 0:w], func=Act.Exp,
                            accum_out=acc[:, seg, t, s:s + 1],
                        )

            # head (seg 2)
            seg_loop(
                2, d_wh, KH, SUPS_HEAD,
                lambda k, t: xT_sb[:, k, t * 128:(t + 1) * 128], 128,
            )
            # tail0 (seg 0)
            seg_loop(
                0, d_w0, K0, SUPS_TAIL,
                lambda k, t: h0T_sb[:, k, t * 128:(t + 1) * 128], 128,
            )
            # tail1 (seg 1): K=64, weights on 64 partitions
            seg_loop(
                1, d_w1, 1, SUPS_TAIL,
                lambda k, t: h1T_sb[0:P1, t * 128:(t + 1) * 128], P1,
            )

            # ---- epilogue: loss = mask * (ln(sumexp) - picked) ----
            for seg in range(3):
                for t in range(TT):
                    c = seg * TT + t
                    nc.vector.tensor_reduce(
                        out=sums[:, c:c + 1], in_=acc[:, seg, t, :],
                        axis=mybir.AxisListType.X, op=Alu.add,
                    )
            nc.scalar.activation(out=lnS[:, :], in_=sums[:, :], func=Act.Ln)
            nc.vector.tensor_sub(loss[:, :], lnS[:, :], picked[:, :])
            nc.vector.tensor_mul(loss[:, 0 * TT:1 * TT], loss[:, 0 * TT:1 * TT], m0_sb[:, :])
            nc.vector.tensor_mul(loss[:, 1 * TT:2 * TT], loss[:, 1 * TT:2 * TT], m1_sb[:, :])
            nc.sync.dma_start(out=d_out.ap()[:, :], in_=loss[:, :])

    nc.compile()
    return nc


def get_nc():
    global _NC_CACHE
    if _NC_CACHE is None:
        _NC_CACHE = _build_nc()
    return _NC_CACHE


def _prep_inputs(inputs, labels, head_weight, tail_proj_0, tail_w_0,
                 tail_proj_1, tail_w_1):
    """Host-side shard + index prep. Returns in_maps (list of 8 dicts)."""
    x = np.asarray(inputs, np.float32).reshape(N, H)
    lab = np.asarray(labels).reshape(N).astype(np.int64)
    wh = np.asarray(head_weight, np.float32)
    p0 = np.asarray(tail_proj_0, np.float32)
    w0 = np.asarray(tail_w_0, np.float32)
    p1 = np.asarray(tail_proj_1, np.float32)
    w1 = np.asarray(tail_w_1, np.float32)

    head_lab = np.where(lab >= CUT1, CUT0 + 1, np.where(lab >= CUT0, CUT0, lab))
    t0_lab = np.clip(lab - CUT0, 0, V_TAIL - 1)
    t1_lab = np.clip(lab - CUT1, 0, V_TAIL - 1)
    m0 = ((lab >= CUT0) & (lab < CUT1)).astype(np.float32)
    m1 = (lab >= CUT1).astype(np.float32)

    wg_all = wh.T[head_lab]      # [N, H]
    w0g_all = w0.T[t0_lab]       # [N, 256]
    w1g_all = w1.T[t1_lab]       # [N, 64]

    # replicated weights (cast once)
    wh_b = np.ascontiguousarray(wh, dtype=BF16)
    w0_b = np.ascontiguousarray(w0, dtype=BF16)
    w1_b = np.ascontiguousarray(w1, dtype=BF16)
    p0_b = np.ascontiguousarray(p0, dtype=BF16)
    p1_b = np.ascontiguousarray(p1, dtype=BF16)

    in_maps = []
    for c in range(NCORES):
        sl = slice(c * TOK, (c + 1) * TOK)
        xc = x[sl]
        in_maps.append({
            "xT": np.ascontiguousarray(xc.T, dtype=BF16),
            "x": np.ascontiguousarray(xc, dtype=BF16),
            "wg": np.ascontiguousarray(wg_all[sl], dtype=BF16),
            "w0g": np.ascontiguousarray(w0g_all[sl], dtype=BF16),
            "w1g": np.ascontiguousarray(w1g_all[sl], dtype=BF16),
            "m0": np.ascontiguousarray(m0[sl].reshape(TT, 128).T),
            "m1": np.ascontiguousarray(m1[sl].reshape(TT, 128).T),
            "wh": wh_b, "w0": w0_b, "w1": w1_b, "p0": p0_b, "p1": p1_b,
        })
    return in_maps


def _assemble(results):
    """results: list of 8 dicts with 'out' [128, 12] -> full [3*N] f32."""
    full = np.empty((3, N), np.float32)
    for c in range(NCORES):
        o = np.asarray(results[c]["out"], np.float32)  # [128, 3*TT]
        for seg in range(3):
            blk = o[:, seg * TT:(seg + 1) * TT]        # [128, TT]
            full[seg, c * TOK:(c + 1) * TOK] = blk.T.reshape(TOK)
    return full.reshape(-1)


def kernel(inputs, labels, head_weight, tail_proj_0, tail_w_0,
           tail_proj_1, tail_w_1):
    from concourse.bass_utils import run_bass_kernel_spmd

    nc = get_nc()
    in_maps = _prep_inputs(inputs, labels, head_weight, tail_proj_0, tail_w_0,
                           tail_proj_1, tail_w_1)
    res = run_bass_kernel_spmd(nc, in_maps, core_ids=list(range(NCORES)))
    return _assemble(res.results)
